# revision 9
# baseline (speedup 1.0000x reference)
"""GCN layer on 8 Trainium2 NeuronCores.

out = relu(D^-1/2 A D^-1/2 x W^T + b), A given as COO edge_index [2, E],
deg = in-degree of destination nodes.

Strategy (destination-sharded, no collectives):
 - Host (integer bookkeeping only): partition destination nodes across 8
   cores balanced by degree; pack nodes into 16-slot "windows" with
   <=256 edges; 2 columns of 128 edge-slots per window; 32 windows per
   PSUM group.  Per group, compact the needed source rows of x into a
   group-local table (<=8192 rows, int16-indexable) and emit per-slot
   index / degree-product / node-slot streams.
 - Device: dma_gather x rows (256B each) into edge-slot staging, scale
   by rsqrt(deg_src*deg_dst) (computed on device) with cast to bf16,
   segment-sum via PE matmuls against device-built selection patterns
   (iota==nid compare), evacuate PSUM, then a replicated 64x64 W matmul
   + bias + ReLU on the transposed aggregate.
 - Host: inverse-permute positions -> nodes, concat cores.
"""
import numpy as np

N_NODES = 100000
N_EDGES = 1600000
D = 64
NCORES = 8

WIN_NODES = 16           # node slots per window
WIN_EDGES = 256          # edge slots per window (2 columns of 128)
WIN_TARGET = 224         # packing target to leave slack
GROUP_WINS = 32          # windows per PSUM group  -> psum [64, 512]
GROUP_COLS = 2 * GROUP_WINS          # 64 columns of 128 slots
GROUP_SLOTS = GROUP_COLS * 128       # 8192 edge slots per group
CALL_IDX = 896           # dma_gather idx per call (56 ring descs, ring-safe)
CALL_COLS = CALL_IDX // 128          # 7 columns per call
PAD_NID = 255.0

_CACHE = {}


# ----------------------------------------------------------------- host pack
def _pack(x, row, col, deg):
    rng_nodes = np.argsort(-deg, kind="stable")       # sorted desc by degree
    core_of = np.empty(N_NODES, np.int32)
    core_of[rng_nodes] = np.arange(N_NODES) % NCORES

    # per-core window assignment (round-robin of globally sorted nodes)
    win_of = np.empty(N_NODES, np.int32)              # core-local window id
    nid_of = np.empty(N_NODES, np.int32)              # slot within window
    nw_per_core = np.zeros(NCORES, np.int64)
    for c in range(NCORES):
        nodes_c = rng_nodes[core_of[rng_nodes] == c]  # still sorted desc
        deg_c = deg[nodes_c]
        nw = max(int(np.ceil(len(nodes_c) / (WIN_NODES - 1))),
                 int(np.ceil(deg_c.sum() / WIN_TARGET)), 1)
        for _ in range(40):
            w = np.arange(len(nodes_c)) % nw
            cnt = np.bincount(w, minlength=nw)
            s = np.bincount(w, weights=deg_c, minlength=nw)
            if cnt.max() <= WIN_NODES and s.max() <= WIN_EDGES:
                break
            nw = int(nw * 1.05) + 1
        else:
            raise RuntimeError("window packing failed")
        win_of[nodes_c] = w
        nid_of[nodes_c] = np.arange(len(nodes_c)) // nw
        nw_per_core[c] = nw

    ng = int(np.ceil(nw_per_core.max() / GROUP_WINS))
    ncol = ng * GROUP_COLS
    calls_per_group = GROUP_SLOTS // CALL_IDX          # 9 full calls
    tail_idx = GROUP_SLOTS - calls_per_group * CALL_IDX  # 128-idx tail call

    # edge placement: position of each edge within its (core, window)
    ecore = core_of[row]
    ewin = win_of[row]
    enid = nid_of[row]
    order = np.lexsort((col, ewin, ecore))
    ecore_s, ewin_s, enid_s, col_s, row_s = (
        ecore[order], ewin[order], enid[order], col[order], row[order])
    key = ecore_s.astype(np.int64) * (2 ** 32) + ewin_s
    starts = np.searchsorted(key, key, side="left")
    pos_in_win = np.arange(len(key)) - starts          # 0..deg(window)-1

    g_of_edge = ewin_s // GROUP_WINS
    wloc = ewin_s % GROUP_WINS
    c_local = 2 * wloc + pos_in_win // 128
    slot = c_local * 128 + (pos_in_win % 128)          # 0..8191 within group

    gkey = ecore_s.astype(np.int64) * (2 ** 32) + g_of_edge
    gorder = np.argsort(gkey, kind="stable")
    gkey_s = gkey[gorder]
    bounds = np.searchsorted(
        gkey_s,
        (np.arange(NCORES, dtype=np.int64)[:, None] * (2 ** 32)
         + np.arange(ng)[None, :]).ravel(), side="left")
    bounds = np.append(bounds, len(gkey_s))

    dp_edge = (deg[col_s] * deg[row_s]).astype(np.float32)

    tabs = np.zeros((NCORES, ng, GROUP_SLOTS, D), np.float32)
    idx_all = np.zeros((NCORES, ng, GROUP_SLOTS), np.int16)
    dp_all = np.zeros((NCORES, ng, GROUP_SLOTS), np.float32)
    nid_all = np.full((NCORES, ng, GROUP_SLOTS), PAD_NID, np.float32)

    for c in range(NCORES):
        for g in range(ng):
            k = c * ng + g
            sel = gorder[bounds[k]:bounds[k + 1]]
            if len(sel) == 0:
                continue
            sslot = slot[sel]
            uniq, inv = np.unique(col_s[sel], return_inverse=True)
            tabs[c, g, :len(uniq)] = x[uniq]
            idx_all[c, g, sslot] = inv.astype(np.int16)
            dp_all[c, g, sslot] = dp_edge[sel]
            nid_all[c, g, sslot] = enid_s[sel]

    # device layouts
    # staging layout: group slot s=(c_local*128+p) <-> call k=s//896, e=s%896
    # per-call wrapped idx: wrapped[p, j] = call_idx[j*16 + p%16]
    slots_lin = idx_all.reshape(NCORES, ng, GROUP_SLOTS)
    idx_sb = np.zeros((NCORES, 128, ncol * 8), np.int16)
    off = 0
    for g in range(ng):
        for k in range(calls_per_group + 1):
            n = CALL_IDX if k < calls_per_group else tail_idx
            if n == 0:
                continue
            blk = slots_lin[:, g, k * CALL_IDX:k * CALL_IDX + n]   # [NC, n]
            wr = blk.reshape(NCORES, n // 16, 16)
            for pg in range(8):
                idx_sb[:, pg * 16:(pg + 1) * 16, off:off + n // 16] = \
                    wr.transpose(0, 2, 1)
            off += n // 16
    assert off == ncol * 8

    # slot-layout streams [128, ncol]: value at (p, 64*g + c_local)
    dp_sb = dp_all.reshape(NCORES, ng, GROUP_COLS, 128).transpose(0, 3, 1, 2) \
        .reshape(NCORES, 128, ncol)
    nid_sb = nid_all.reshape(NCORES, ng, GROUP_COLS, 128).transpose(0, 3, 1, 2) \
        .reshape(NCORES, 128, ncol)

    # output position of each node: core, 16*ewin + nid
    outpos = 16 * win_of.astype(np.int64) + nid_of
    return dict(ng=ng, ncol=ncol, tabs=tabs, idx_sb=idx_sb, dp_sb=dp_sb,
                nid_sb=nid_sb, core_of=core_of, outpos=outpos)


# ------------------------------------------------------------- device kernel
def _build(ngc):
    import concourse.bass as bass
    import concourse.bacc as bacc
    import concourse.mybir as mybir
    from concourse.tile import TileContext
    from concourse.library_config import mlp
    from concourse.tile_rust import add_dep_helper

    dt = mybir.dt
    ng = ngc
    ncol = ng * GROUP_COLS
    calls_per_group = GROUP_SLOTS // CALL_IDX
    tail_idx = GROUP_SLOTS - calls_per_group * CALL_IDX

    nc = bacc.Bacc("TRN2", target_bir_lowering=False, debug=False,
                   num_devices=NCORES)
    tab_d = nc.dram_tensor("tab", [ng, GROUP_SLOTS, D], dt.float32,
                           kind="ExternalInput")
    idx_d = nc.dram_tensor("idx", [128, ncol * 8], dt.int16, kind="ExternalInput")
    dp_d = nc.dram_tensor("dp", [128, ncol], dt.float32, kind="ExternalInput")
    nid_d = nc.dram_tensor("nid", [128, ncol], dt.bfloat16, kind="ExternalInput")
    iota_d = nc.dram_tensor("iota", [128, 16], dt.bfloat16, kind="ExternalInput")
    wt_d = nc.dram_tensor("wt", [64, 64], dt.float32, kind="ExternalInput")
    b_d = nc.dram_tensor("bias", [64, 1], dt.float32, kind="ExternalInput")
    out_d = nc.dram_tensor("out", [64, ng * 512], dt.float32, kind="ExternalOutput")

    with TileContext(nc) as tc:
        with (
            tc.tile_pool(name="fixed", bufs=1) as fixed_pool,
            tc.tile_pool(name="stg", bufs=2) as stg_pool,
            tc.tile_pool(name="sbf", bufs=3) as sbf_pool,
            tc.tile_pool(name="pat", bufs=3) as pat_pool,
            tc.tile_pool(name="otile", bufs=3) as o_pool,
            tc.tile_pool(name="psum", bufs=4, space="PSUM") as psum_pool,
            tc.tile_pool(name="psum2", bufs=2, space="PSUM") as psum2_pool,
        ):
            idx_sb = fixed_pool.tile([128, ncol * 8], dt.int16, tag="idx_sb")
            dis_sb = fixed_pool.tile([128, ncol], dt.float32, tag="dis_sb")
            nid_sb = fixed_pool.tile([128, ncol], dt.bfloat16, tag="nid_sb")
            iota_sb = fixed_pool.tile([128, 16], dt.bfloat16, tag="iota_sb")
            wt_sb = fixed_pool.tile([64, 64], dt.float32, tag="wt_sb")
            wtb_sb = fixed_pool.tile([64, 64], dt.bfloat16, tag="wtb_sb")
            b_sb = fixed_pool.tile([64, 1], dt.float32, tag="b_sb")
            agg_sb = fixed_pool.tile([64, ng * 512], dt.bfloat16, tag="agg_sb")
            t1 = fixed_pool.tile([128, ncol], dt.float32, tag="t1")
            t2 = fixed_pool.tile([128, ncol], dt.float32, tag="t2")

            lib_inst = nc.gpsimd.load_library(mlp)

            nc.sync.dma_start(out=idx_sb[:], in_=idx_d[:])
            nc.sync.dma_start(out=t1[:], in_=dp_d[:])
            nc.sync.dma_start(out=nid_sb[:], in_=nid_d[:])
            nc.sync.dma_start(out=iota_sb[:], in_=iota_d[:])
            nc.sync.dma_start(out=wt_sb[:], in_=wt_d[:])
            nc.sync.dma_start(out=b_sb[:], in_=b_d[:])

            # dis = rsqrt(max(dp,1)) * min(dp,1)   (0 where dp==0)
            nc.vector.tensor_scalar(out=t2[:], in0=t1[:], scalar1=1.0,
                                    scalar2=None, op0=mybir.AluOpType.max)
            nc.scalar.activation(t2[:], t2[:], mybir.ActivationFunctionType.Sqrt)
            nc.vector.reciprocal(t2[:], t2[:])
            nc.vector.tensor_scalar(out=t1[:], in0=t1[:], scalar1=1.0,
                                    scalar2=None, op0=mybir.AluOpType.min)
            nc.vector.tensor_tensor(out=dis_sb[:], in0=t1[:], in1=t2[:],
                                    op=mybir.AluOpType.mult)
            nc.vector.tensor_copy(out=wtb_sb[:], in_=wt_sb[:])

            first_gather = [None]

            for g in range(ng):
                # selection pattern for the whole group: [128, 64*16] bf16
                patt = pat_pool.tile([128, GROUP_COLS, 16], dt.bfloat16)
                iota_rep = bass.AP(iota_sb[:].tensor, iota_sb[:].offset,
                                   [iota_sb[:].ap[0], [0, GROUP_COLS],
                                    iota_sb[:].ap[1]])
                nid_slice = nid_sb[:, g * GROUP_COLS:(g + 1) * GROUP_COLS]
                nid_b = bass.AP(nid_slice.tensor, nid_slice.offset,
                                [nid_slice.ap[0], nid_slice.ap[1], [0, 16]])
                nc.vector.tensor_tensor(out=patt[:], in0=iota_rep, in1=nid_b,
                                        op=mybir.AluOpType.is_equal)

                psum_t = psum_pool.tile([64, 512], dt.float32, space="PSUM")
                idx_off = g * 512
                ncalls = calls_per_group + (1 if tail_idx else 0)
                for k in range(ncalls):
                    nidx = CALL_IDX if k < calls_per_group else tail_idx
                    cols = nidx // 128
                    stg = stg_pool.tile([128, CALL_COLS, D], dt.float32)
                    gi = nc.gpsimd.dma_gather(
                        stg[:, :cols, :], tab_d[g, :, :],
                        idx_sb[:, idx_off:idx_off + nidx // 16],
                        nidx, nidx, D)
                    if first_gather[0] is None:
                        first_gather[0] = gi
                        add_dep_helper(gi.ins, lib_inst.ins, False)
                    idx_off += nidx // 16

                    # scale by dis and cast to bf16
                    sbf = sbf_pool.tile([128, CALL_COLS, D], dt.bfloat16)
                    c0 = g * GROUP_COLS + k * CALL_COLS
                    dsl = dis_sb[:, c0:c0 + cols]
                    dis_b = bass.AP(dsl.tensor, dsl.offset,
                                    [dsl.ap[0], dsl.ap[1], [0, D]])
                    nc.vector.tensor_tensor(out=sbf[:, :cols, :],
                                            in0=stg[:, :cols, :], in1=dis_b,
                                            op=mybir.AluOpType.mult)
                    for i in range(cols):
                        cl = k * CALL_COLS + i
                        w = cl >> 1
                        nc.tensor.matmul(
                            out=psum_t[:, 16 * w:16 * w + 16],
                            lhsT=sbf[:, i, :],
                            rhs=patt[:, cl, :],
                            start=(cl & 1) == 0, stop=(cl & 1) == 1)

                nc.scalar.activation(agg_sb[:, g * 512:(g + 1) * 512], psum_t[:],
                                     mybir.ActivationFunctionType.Copy)

            for g in range(ng):
                ps2 = psum2_pool.tile([64, 512], dt.float32, space="PSUM")
                nc.tensor.matmul(out=ps2[:], lhsT=wtb_sb[:],
                                 rhs=agg_sb[:, g * 512:(g + 1) * 512],
                                 start=True, stop=True)
                ot = o_pool.tile([64, 512], dt.float32)
                nc.scalar.activation(ot[:], ps2[:],
                                     mybir.ActivationFunctionType.Relu,
                                     bias=b_sb[:])
                nc.sync.dma_start(out=out_d[:, g * 512:(g + 1) * 512], in_=ot[:])

    nc.compile()
    return nc


# ------------------------------------------------------------------- runner
def _make_runner(nc):
    import jax
    import numpy as _np
    import concourse.mybir as mybir
    from concourse.bass2jax import _bass_exec_p, install_neuronx_cc_hook
    from jax.sharding import Mesh, PartitionSpec
    from jax.experimental.shard_map import shard_map

    install_neuronx_cc_hook()
    in_names, out_names, out_avals, zero_outs = [], [], [], []
    for alloc in nc.m.functions[0].allocations:
        if not isinstance(alloc, mybir.MemoryLocationSet):
            continue
        name = alloc.memorylocations[0].name
        if alloc.kind == "ExternalInput":
            in_names.append(name)
        elif alloc.kind == "ExternalOutput":
            out_names.append(name)
            shape = tuple(alloc.tensor_shape)
            out_avals.append(jax.core.ShapedArray(shape, mybir.dt.np(alloc.dtype)))
            zero_outs.append(_np.zeros(shape, mybir.dt.np(alloc.dtype)))
    all_names = in_names + out_names

    def _body(*args):
        return tuple(_bass_exec_p.bind(
            *args, out_avals=tuple(out_avals), in_names=tuple(all_names),
            out_names=tuple(out_names), lowering_input_output_aliases=(),
            sim_require_finite=True, sim_require_nnan=True, nc=nc))

    devices = jax.devices()[:NCORES]
    mesh = Mesh(np.asarray(devices), ("core",))
    fn = jax.jit(
        shard_map(_body, mesh=mesh,
                  in_specs=(PartitionSpec("core"),) * (len(in_names) + len(out_names)),
                  out_specs=(PartitionSpec("core"),) * len(out_names),
                  check_rep=False),
        keep_unused=True)
    return fn, in_names, out_names, zero_outs, mesh


def _feeds(packed, W, b):
    import ml_dtypes
    ng, ncol = packed["ng"], packed["ncol"]
    iota = np.broadcast_to(np.arange(16, dtype=np.float32), (128, 16))
    return {
        "tab": packed["tabs"],
        "idx": packed["idx_sb"],
        "dp": packed["dp_sb"],
        "nid": packed["nid_sb"].astype(ml_dtypes.bfloat16),
        "iota": np.broadcast_to(iota.astype(ml_dtypes.bfloat16),
                                (NCORES, 128, 16)),
        "wt": np.broadcast_to(np.ascontiguousarray(W.T), (NCORES, 64, 64)),
        "bias": np.broadcast_to(b.reshape(64, 1), (NCORES, 64, 1)),
        "partition_id": np.arange(NCORES, dtype=np.uint32).reshape(NCORES, 1, 1),
    }


NGC = 7   # groups per NEFF execution (keeps per-engine inst count < 4096)


def run_device(packed, W, b):
    import jax
    from jax.sharding import NamedSharding, PartitionSpec
    ng = packed["ng"]
    nchunks = -(-ng // NGC)
    key = ("nc", NGC)
    if key not in _CACHE:
        nc = _build(NGC)
        _CACHE[key] = _make_runner(nc)
    fn, in_names, out_names, zero_outs, mesh = _CACHE[key]
    feeds = _feeds(packed, W, b)
    shard = NamedSharding(mesh, PartitionSpec("core"))
    oi = out_names.index("out")

    # pad feeds to nchunks*NGC groups along their group axes
    ngp = nchunks * NGC
    tabs = np.zeros((NCORES, ngp) + feeds["tab"].shape[2:], np.float32)
    tabs[:, :ng] = feeds["tab"]
    idxf = np.zeros((NCORES, 128, ngp * 512), np.int16)
    idxf[:, :, :ng * 512] = feeds["idx"]
    dpf = np.zeros((NCORES, 128, ngp * 64), np.float32)
    dpf[:, :, :ng * 64] = feeds["dp"]
    nidf = np.full((NCORES, 128, ngp * 64), PAD_NID, feeds["nid"].dtype)
    nidf[:, :, :ng * 64] = feeds["nid"]

    res = np.empty((NCORES, 64, ngp * 512), np.float32)
    devs = []
    for ch in range(nchunks):
        g0, g1 = ch * NGC, (ch + 1) * NGC
        chunk_feeds = dict(feeds)
        chunk_feeds["tab"] = tabs[:, g0:g1]
        chunk_feeds["idx"] = idxf[:, :, g0 * 512:g1 * 512]
        chunk_feeds["dp"] = dpf[:, :, g0 * 64:g1 * 64]
        chunk_feeds["nid"] = nidf[:, :, g0 * 64:g1 * 64]
        concat = []
        for nm in in_names:
            v = np.ascontiguousarray(np.asarray(chunk_feeds[nm]))
            concat.append(np.concatenate([v[c] for c in range(NCORES)], axis=0))
        concat += [np.concatenate([z] * NCORES, axis=0) for z in zero_outs]
        devs.append([jax.device_put(a, shard) for a in concat])
    all_outs = [fn(*d) for d in devs]
    jax.block_until_ready(all_outs)
    for ch, outs in enumerate(all_outs):
        res[:, :, ch * NGC * 512:(ch + 1) * NGC * 512] = \
            np.asarray(outs[oi]).reshape(NCORES, *zero_outs[oi].shape)
    _CACHE["last_exec"] = (fn, devs)
    return res[:, :, :ng * 512]


def kernel(x, edge_index, W, b):
    x = np.asarray(x, dtype=np.float32)
    edge_index = np.asarray(edge_index)
    W = np.asarray(W, dtype=np.float32)
    b = np.asarray(b, dtype=np.float32)
    row = edge_index[0].astype(np.int64)
    col = edge_index[1].astype(np.int64)
    deg = np.bincount(row, minlength=N_NODES).astype(np.int64)
    assert deg.max() <= WIN_EDGES, "node degree exceeds window capacity"

    packed = _pack(x, row, col, deg)
    res = run_device(packed, W, b)            # [NC, 64, ng*512]

    out = np.empty((N_NODES, D), np.float32)
    core_of, outpos = packed["core_of"], packed["outpos"]
    for c in range(NCORES):
        nodes = np.flatnonzero(core_of == c)
        out[nodes] = res[c][:, outpos[nodes]].T
    return out


# revision 10
# speedup vs baseline: 2.5961x; 2.5961x over previous
"""GCN layer on 8 Trainium2 NeuronCores.

out = relu(D^-1/2 A D^-1/2 x W^T + b), A given as COO edge_index [2, E],
deg = in-degree of destination nodes.

Strategy (destination-sharded, no collectives):
 - Host (integer bookkeeping only): partition destination nodes across 8
   cores balanced by degree; pack nodes into 16-slot "windows" with
   <=256 edges; 2 columns of 128 edge-slots per window; 32 windows per
   PSUM group.  Per group, compact the needed source rows of x into a
   group-local table (<=8192 rows, int16-indexable) and emit per-slot
   index / degree-product / node-slot streams.
 - Device: dma_gather x rows (256B each) into edge-slot staging, scale
   by rsqrt(deg_src*deg_dst) (computed on device) with cast to bf16,
   segment-sum via PE matmuls against device-built selection patterns
   (iota==nid compare), evacuate PSUM, then a replicated 64x64 W matmul
   + bias + ReLU on the transposed aggregate.
 - Host: inverse-permute positions -> nodes, concat cores.
"""
import numpy as np

N_NODES = 100000
N_EDGES = 1600000
D = 64
NCORES = 8

WIN_NODES = 16           # node slots per window
WIN_EDGES = 256          # edge slots per window (2 columns of 128)
WIN_TARGET = 224         # packing target to leave slack
GROUP_WINS = 32          # windows per PSUM group  -> psum [64, 512]
GROUP_COLS = 2 * GROUP_WINS          # 64 columns of 128 slots
GROUP_SLOTS = GROUP_COLS * 128       # 8192 edge slots per group
CALL_IDX = 896           # dma_gather idx per call (56 ring descs, ring-safe)
CALL_COLS = CALL_IDX // 128          # 7 columns per call
PAD_NID = 255.0

_CACHE = {}


# ----------------------------------------------------------------- host pack
def _pack(x, row, col, deg):
    rng_nodes = np.argsort(-deg, kind="stable")       # sorted desc by degree
    core_of = np.empty(N_NODES, np.int32)
    core_of[rng_nodes] = np.arange(N_NODES) % NCORES

    # per-core window assignment (round-robin of globally sorted nodes)
    win_of = np.empty(N_NODES, np.int32)              # core-local window id
    nid_of = np.empty(N_NODES, np.int32)              # slot within window
    nw_per_core = np.zeros(NCORES, np.int64)
    for c in range(NCORES):
        nodes_c = rng_nodes[core_of[rng_nodes] == c]  # still sorted desc
        deg_c = deg[nodes_c]
        nw = max(int(np.ceil(len(nodes_c) / (WIN_NODES - 1))),
                 int(np.ceil(deg_c.sum() / WIN_TARGET)), 1)
        for _ in range(40):
            w = np.arange(len(nodes_c)) % nw
            cnt = np.bincount(w, minlength=nw)
            s = np.bincount(w, weights=deg_c, minlength=nw)
            if cnt.max() <= WIN_NODES and s.max() <= WIN_EDGES:
                break
            nw = int(nw * 1.05) + 1
        else:
            raise RuntimeError("window packing failed")
        win_of[nodes_c] = w
        nid_of[nodes_c] = np.arange(len(nodes_c)) // nw
        nw_per_core[c] = nw

    ng = int(np.ceil(nw_per_core.max() / GROUP_WINS))
    ncol = ng * GROUP_COLS
    calls_per_group = GROUP_SLOTS // CALL_IDX          # 9 full calls
    tail_idx = GROUP_SLOTS - calls_per_group * CALL_IDX  # 128-idx tail call

    # edge placement: position of each edge within its (core, window)
    ecore = core_of[row]
    ewin = win_of[row]
    enid = nid_of[row]
    order = np.lexsort((col, ewin, ecore))
    ecore_s, ewin_s, enid_s, col_s, row_s = (
        ecore[order], ewin[order], enid[order], col[order], row[order])
    key = ecore_s.astype(np.int64) * (2 ** 32) + ewin_s
    starts = np.searchsorted(key, key, side="left")
    pos_in_win = np.arange(len(key)) - starts          # 0..deg(window)-1

    g_of_edge = ewin_s // GROUP_WINS
    wloc = ewin_s % GROUP_WINS
    c_local = 2 * wloc + pos_in_win // 128
    slot = c_local * 128 + (pos_in_win % 128)          # 0..8191 within group

    gkey = ecore_s.astype(np.int64) * (2 ** 32) + g_of_edge
    gorder = np.argsort(gkey, kind="stable")
    gkey_s = gkey[gorder]
    bounds = np.searchsorted(
        gkey_s,
        (np.arange(NCORES, dtype=np.int64)[:, None] * (2 ** 32)
         + np.arange(ng)[None, :]).ravel(), side="left")
    bounds = np.append(bounds, len(gkey_s))

    dp_edge = (deg[col_s] * deg[row_s]).astype(np.float32)

    tabs = np.zeros((NCORES, ng, GROUP_SLOTS, D), np.float32)
    idx_all = np.zeros((NCORES, ng, GROUP_SLOTS), np.int16)
    dp_all = np.zeros((NCORES, ng, GROUP_SLOTS), np.float32)
    nid_all = np.full((NCORES, ng, GROUP_SLOTS), PAD_NID, np.float32)

    for c in range(NCORES):
        for g in range(ng):
            k = c * ng + g
            sel = gorder[bounds[k]:bounds[k + 1]]
            if len(sel) == 0:
                continue
            sslot = slot[sel]
            uniq, inv = np.unique(col_s[sel], return_inverse=True)
            tabs[c, g, :len(uniq)] = x[uniq]
            idx_all[c, g, sslot] = inv.astype(np.int16)
            dp_all[c, g, sslot] = dp_edge[sel]
            nid_all[c, g, sslot] = enid_s[sel]

    # device layouts
    # staging layout: group slot s=(c_local*128+p) <-> call k=s//896, e=s%896
    # per-call wrapped idx: wrapped[p, j] = call_idx[j*16 + p%16]
    slots_lin = idx_all.reshape(NCORES, ng, GROUP_SLOTS)
    idx_sb = np.zeros((NCORES, 128, ncol * 8), np.int16)
    off = 0
    for g in range(ng):
        for k in range(calls_per_group + 1):
            n = CALL_IDX if k < calls_per_group else tail_idx
            if n == 0:
                continue
            blk = slots_lin[:, g, k * CALL_IDX:k * CALL_IDX + n]   # [NC, n]
            wr = blk.reshape(NCORES, n // 16, 16)
            for pg in range(8):
                idx_sb[:, pg * 16:(pg + 1) * 16, off:off + n // 16] = \
                    wr.transpose(0, 2, 1)
            off += n // 16
    assert off == ncol * 8

    # slot-layout streams [128, ncol]: value at (p, 64*g + c_local)
    dp_sb = dp_all.reshape(NCORES, ng, GROUP_COLS, 128).transpose(0, 3, 1, 2) \
        .reshape(NCORES, 128, ncol)
    nid_sb = nid_all.reshape(NCORES, ng, GROUP_COLS, 128).transpose(0, 3, 1, 2) \
        .reshape(NCORES, 128, ncol)

    # output position of each node: core, 16*ewin + nid
    outpos = 16 * win_of.astype(np.int64) + nid_of
    return dict(ng=ng, ncol=ncol, tabs=tabs, idx_sb=idx_sb, dp_sb=dp_sb,
                nid_sb=nid_sb, core_of=core_of, outpos=outpos)


# ------------------------------------------------------------- device kernel
def _build(ngc):
    import concourse.bass as bass
    import concourse.bacc as bacc
    import concourse.mybir as mybir
    from concourse.tile import TileContext
    from concourse.library_config import mlp
    from concourse.tile_rust import add_dep_helper

    dt = mybir.dt
    ng = ngc
    ncol = ng * GROUP_COLS
    calls_per_group = GROUP_SLOTS // CALL_IDX
    tail_idx = GROUP_SLOTS - calls_per_group * CALL_IDX

    nc = bacc.Bacc("TRN2", target_bir_lowering=False, debug=False,
                   num_devices=NCORES)
    tab_d = nc.dram_tensor("tab", [ng, GROUP_SLOTS, D], dt.float32,
                           kind="ExternalInput")
    idx_d = nc.dram_tensor("idx", [128, ncol * 8], dt.int16, kind="ExternalInput")
    dp_d = nc.dram_tensor("dp", [128, ncol], dt.float32, kind="ExternalInput")
    nid_d = nc.dram_tensor("nid", [128, ncol], dt.bfloat16, kind="ExternalInput")
    iota_d = nc.dram_tensor("iota", [128, 16], dt.bfloat16, kind="ExternalInput")
    wt_d = nc.dram_tensor("wt", [64, 64], dt.float32, kind="ExternalInput")
    b_d = nc.dram_tensor("bias", [64, 1], dt.float32, kind="ExternalInput")
    out_d = nc.dram_tensor("out", [64, ng * 512], dt.float32, kind="ExternalOutput")

    with TileContext(nc) as tc:
        with (
            tc.tile_pool(name="fixed", bufs=1) as fixed_pool,
            tc.tile_pool(name="stg", bufs=2) as stg_pool,
            tc.tile_pool(name="sbf", bufs=3) as sbf_pool,
            tc.tile_pool(name="pat", bufs=3) as pat_pool,
            tc.tile_pool(name="otile", bufs=3) as o_pool,
            tc.tile_pool(name="psum", bufs=4, space="PSUM") as psum_pool,
            tc.tile_pool(name="psum2", bufs=2, space="PSUM") as psum2_pool,
        ):
            idx_sb = fixed_pool.tile([128, ncol * 8], dt.int16, tag="idx_sb")
            dis_sb = fixed_pool.tile([128, ncol], dt.float32, tag="dis_sb")
            nid_sb = fixed_pool.tile([128, ncol], dt.bfloat16, tag="nid_sb")
            iota_sb = fixed_pool.tile([128, 16], dt.bfloat16, tag="iota_sb")
            wt_sb = fixed_pool.tile([64, 64], dt.float32, tag="wt_sb")
            wtb_sb = fixed_pool.tile([64, 64], dt.bfloat16, tag="wtb_sb")
            b_sb = fixed_pool.tile([64, 1], dt.float32, tag="b_sb")
            agg_sb = fixed_pool.tile([64, ng * 512], dt.bfloat16, tag="agg_sb")
            t1 = fixed_pool.tile([128, ncol], dt.float32, tag="t1")
            t2 = fixed_pool.tile([128, ncol], dt.float32, tag="t2")

            lib_inst = nc.gpsimd.load_library(mlp)

            nc.sync.dma_start(out=idx_sb[:], in_=idx_d[:])
            nc.sync.dma_start(out=t1[:], in_=dp_d[:])
            nc.sync.dma_start(out=nid_sb[:], in_=nid_d[:])
            nc.sync.dma_start(out=iota_sb[:], in_=iota_d[:])
            nc.sync.dma_start(out=wt_sb[:], in_=wt_d[:])
            nc.sync.dma_start(out=b_sb[:], in_=b_d[:])

            # dis = rsqrt(max(dp,1)) * min(dp,1)   (0 where dp==0)
            nc.vector.tensor_scalar(out=t2[:], in0=t1[:], scalar1=1.0,
                                    scalar2=None, op0=mybir.AluOpType.max)
            nc.scalar.activation(t2[:], t2[:], mybir.ActivationFunctionType.Sqrt)
            nc.vector.reciprocal(t2[:], t2[:])
            nc.vector.tensor_scalar(out=t1[:], in0=t1[:], scalar1=1.0,
                                    scalar2=None, op0=mybir.AluOpType.min)
            nc.vector.tensor_tensor(out=dis_sb[:], in0=t1[:], in1=t2[:],
                                    op=mybir.AluOpType.mult)
            nc.vector.tensor_copy(out=wtb_sb[:], in_=wt_sb[:])

            first_gather = [None]

            for g in range(ng):
                # selection pattern for the whole group: [128, 64*16] bf16
                patt = pat_pool.tile([128, GROUP_COLS, 16], dt.bfloat16)
                iota_rep = bass.AP(iota_sb[:].tensor, iota_sb[:].offset,
                                   [iota_sb[:].ap[0], [0, GROUP_COLS],
                                    iota_sb[:].ap[1]])
                nid_slice = nid_sb[:, g * GROUP_COLS:(g + 1) * GROUP_COLS]
                nid_b = bass.AP(nid_slice.tensor, nid_slice.offset,
                                [nid_slice.ap[0], nid_slice.ap[1], [0, 16]])
                nc.vector.tensor_tensor(out=patt[:], in0=iota_rep, in1=nid_b,
                                        op=mybir.AluOpType.is_equal)

                psum_t = psum_pool.tile([64, 512], dt.float32, space="PSUM")
                idx_off = g * 512
                ncalls = calls_per_group + (1 if tail_idx else 0)
                for k in range(ncalls):
                    nidx = CALL_IDX if k < calls_per_group else tail_idx
                    cols = nidx // 128
                    stg = stg_pool.tile([128, CALL_COLS, D], dt.float32)
                    gi = nc.gpsimd.dma_gather(
                        stg[:, :cols, :], tab_d[g, :, :],
                        idx_sb[:, idx_off:idx_off + nidx // 16],
                        nidx, nidx, D)
                    if first_gather[0] is None:
                        first_gather[0] = gi
                        add_dep_helper(gi.ins, lib_inst.ins, False)
                    idx_off += nidx // 16

                    # scale by dis and cast to bf16
                    sbf = sbf_pool.tile([128, CALL_COLS, D], dt.bfloat16)
                    c0 = g * GROUP_COLS + k * CALL_COLS
                    dsl = dis_sb[:, c0:c0 + cols]
                    dis_b = bass.AP(dsl.tensor, dsl.offset,
                                    [dsl.ap[0], dsl.ap[1], [0, D]])
                    nc.vector.tensor_tensor(out=sbf[:, :cols, :],
                                            in0=stg[:, :cols, :], in1=dis_b,
                                            op=mybir.AluOpType.mult)
                    for i in range(cols):
                        cl = k * CALL_COLS + i
                        w = cl >> 1
                        nc.tensor.matmul(
                            out=psum_t[:, 16 * w:16 * w + 16],
                            lhsT=sbf[:, i, :],
                            rhs=patt[:, cl, :],
                            start=(cl & 1) == 0, stop=(cl & 1) == 1)

                nc.scalar.activation(agg_sb[:, g * 512:(g + 1) * 512], psum_t[:],
                                     mybir.ActivationFunctionType.Copy)

            for g in range(ng):
                ps2 = psum2_pool.tile([64, 512], dt.float32, space="PSUM")
                nc.tensor.matmul(out=ps2[:], lhsT=wtb_sb[:],
                                 rhs=agg_sb[:, g * 512:(g + 1) * 512],
                                 start=True, stop=True)
                ot = o_pool.tile([64, 512], dt.float32)
                nc.scalar.activation(ot[:], ps2[:],
                                     mybir.ActivationFunctionType.Relu,
                                     bias=b_sb[:])
                nc.sync.dma_start(out=out_d[:, g * 512:(g + 1) * 512], in_=ot[:])

    nc.compile()
    return nc


# ------------------------------------------------------------------- runner
def _make_runner(nc):
    import jax
    import numpy as _np
    import concourse.mybir as mybir
    from concourse.bass2jax import _bass_exec_p, install_neuronx_cc_hook
    from jax.sharding import Mesh, PartitionSpec
    from jax.experimental.shard_map import shard_map

    install_neuronx_cc_hook()
    in_names, out_names, out_avals, zero_outs = [], [], [], []
    for alloc in nc.m.functions[0].allocations:
        if not isinstance(alloc, mybir.MemoryLocationSet):
            continue
        name = alloc.memorylocations[0].name
        if alloc.kind == "ExternalInput":
            in_names.append(name)
        elif alloc.kind == "ExternalOutput":
            out_names.append(name)
            shape = tuple(alloc.tensor_shape)
            out_avals.append(jax.core.ShapedArray(shape, mybir.dt.np(alloc.dtype)))
            zero_outs.append(_np.zeros(shape, mybir.dt.np(alloc.dtype)))
    all_names = in_names + out_names

    def _body(*args):
        return tuple(_bass_exec_p.bind(
            *args, out_avals=tuple(out_avals), in_names=tuple(all_names),
            out_names=tuple(out_names), lowering_input_output_aliases=(),
            sim_require_finite=True, sim_require_nnan=True, nc=nc))

    devices = jax.devices()[:NCORES]
    mesh = Mesh(np.asarray(devices), ("core",))
    fn = jax.jit(
        shard_map(_body, mesh=mesh,
                  in_specs=(PartitionSpec("core"),) * (len(in_names) + len(out_names)),
                  out_specs=(PartitionSpec("core"),) * len(out_names),
                  check_rep=False),
        keep_unused=True)
    return fn, in_names, out_names, zero_outs, mesh


def _feeds(packed, W, b):
    import ml_dtypes
    ng, ncol = packed["ng"], packed["ncol"]
    iota = np.broadcast_to(np.arange(16, dtype=np.float32), (128, 16))
    return {
        "tab": packed["tabs"],
        "idx": packed["idx_sb"],
        "dp": packed["dp_sb"],
        "nid": packed["nid_sb"].astype(ml_dtypes.bfloat16),
        "iota": np.broadcast_to(iota.astype(ml_dtypes.bfloat16),
                                (NCORES, 128, 16)),
        "wt": np.broadcast_to(np.ascontiguousarray(W.T), (NCORES, 64, 64)),
        "bias": np.broadcast_to(b.reshape(64, 1), (NCORES, 64, 1)),
        "partition_id": np.arange(NCORES, dtype=np.uint32).reshape(NCORES, 1, 1),
    }


NGC = 28  # groups per NEFF execution


def run_device(packed, W, b):
    import jax
    from jax.sharding import NamedSharding, PartitionSpec
    ng = packed["ng"]
    nchunks = -(-ng // NGC)
    key = ("nc", NGC)
    if key not in _CACHE:
        nc = _build(NGC)
        _CACHE[key] = _make_runner(nc)
    fn, in_names, out_names, zero_outs, mesh = _CACHE[key]
    feeds = _feeds(packed, W, b)
    shard = NamedSharding(mesh, PartitionSpec("core"))
    oi = out_names.index("out")

    # pad feeds to nchunks*NGC groups along their group axes
    ngp = nchunks * NGC
    tabs = np.zeros((NCORES, ngp) + feeds["tab"].shape[2:], np.float32)
    tabs[:, :ng] = feeds["tab"]
    idxf = np.zeros((NCORES, 128, ngp * 512), np.int16)
    idxf[:, :, :ng * 512] = feeds["idx"]
    dpf = np.zeros((NCORES, 128, ngp * 64), np.float32)
    dpf[:, :, :ng * 64] = feeds["dp"]
    nidf = np.full((NCORES, 128, ngp * 64), PAD_NID, feeds["nid"].dtype)
    nidf[:, :, :ng * 64] = feeds["nid"]

    res = np.empty((NCORES, 64, ngp * 512), np.float32)
    devs = []
    for ch in range(nchunks):
        g0, g1 = ch * NGC, (ch + 1) * NGC
        chunk_feeds = dict(feeds)
        chunk_feeds["tab"] = tabs[:, g0:g1]
        chunk_feeds["idx"] = idxf[:, :, g0 * 512:g1 * 512]
        chunk_feeds["dp"] = dpf[:, :, g0 * 64:g1 * 64]
        chunk_feeds["nid"] = nidf[:, :, g0 * 64:g1 * 64]
        concat = []
        for nm in in_names:
            v = np.ascontiguousarray(np.asarray(chunk_feeds[nm]))
            concat.append(np.concatenate([v[c] for c in range(NCORES)], axis=0))
        concat += [np.concatenate([z] * NCORES, axis=0) for z in zero_outs]
        devs.append([jax.device_put(a, shard) for a in concat])
    all_outs = [fn(*d) for d in devs]
    jax.block_until_ready(all_outs)
    for ch, outs in enumerate(all_outs):
        res[:, :, ch * NGC * 512:(ch + 1) * NGC * 512] = \
            np.asarray(outs[oi]).reshape(NCORES, *zero_outs[oi].shape)
    _CACHE["last_exec"] = (fn, devs)
    return res[:, :, :ng * 512]


def kernel(x, edge_index, W, b):
    x = np.asarray(x, dtype=np.float32)
    edge_index = np.asarray(edge_index)
    W = np.asarray(W, dtype=np.float32)
    b = np.asarray(b, dtype=np.float32)
    row = edge_index[0].astype(np.int64)
    col = edge_index[1].astype(np.int64)
    deg = np.bincount(row, minlength=N_NODES).astype(np.int64)
    assert deg.max() <= WIN_EDGES, "node degree exceeds window capacity"

    packed = _pack(x, row, col, deg)
    res = run_device(packed, W, b)            # [NC, 64, ng*512]

    out = np.empty((N_NODES, D), np.float32)
    core_of, outpos = packed["core_of"], packed["outpos"]
    for c in range(NCORES):
        nodes = np.flatnonzero(core_of == c)
        out[nodes] = res[c][:, outpos[nodes]].T
    return out
